# revision 17
# baseline (speedup 1.0000x reference)
import sys, os
for p in ("/opt/trn_rl_repo", "/root/.axon_site/_ro/trn_rl_repo"):
    if os.path.isdir(p) and p not in sys.path:
        sys.path.insert(0, p)

import numpy as np
import ml_dtypes

from concourse import bass, bacc, tile
from concourse.bass_utils import run_bass_kernel_spmd

mybir = bass.mybir
BF16 = ml_dtypes.bfloat16
_f32 = mybir.dt.float32
_bf16 = mybir.dt.bfloat16

# Problem constants (hardcoded per contract)
N_NODES = 10000
N_EDGES = 160000
C = 16
Q = 5            # 2*order+1, order=2
CQ = C * Q       # 80
FR = 10          # 5 freqs * 2 rings
OP = C * Q       # 80
NCORES = 8
NPC = N_NODES // NCORES   # 1250 nodes per core
NL = 8           # max nodes per 128-edge tile
EC = 128         # edges per tile
SW = NL * FR     # 80 S columns per tile
NS = 7           # nonlin samples
ORDER = 2
CHUNK_T = 45     # tiles per node-chunk (9 groups of GT=5)
BATCH_T = 25     # tiles per S batch / xt scheduling unit
GT = 5           # tiles per psum group

# S-batch build strategy: batches 0,1,3,5 DMA'd dense; 2,4,6 by gpsimd
# scatter (2.6us each) interleaved with the DMA stream so neither paces
# PE. DVE does staging only.
DENSE_B = (0, 1, 3, 5)
SCAT_B = (2, 4, 6)

_NC_CACHE = {}
_PREP_CACHE = {}
_SD_CACHE = {}
LAST_EXEC_NS = []


# ----------------------------------------------------------------------------
# host helpers
# ----------------------------------------------------------------------------

def _nonlin_np(y):
    """Fourier -> NS regular samples -> ReLU -> Fourier. y: [N,C,Q] f32."""
    theta = (2.0 * np.pi / NS) * np.arange(NS, dtype=np.float32)
    m = np.arange(1, ORDER + 1, dtype=np.float32)
    ang = theta[:, None] * m[None, :]
    cs = np.stack([np.cos(ang), np.sin(ang)], axis=-1).reshape(NS, 2 * ORDER)
    B = np.concatenate([np.ones((NS, 1), np.float32), cs], axis=1).astype(np.float32)
    scale = np.concatenate([np.full((1,), 1.0 / NS, np.float32),
                            np.full((2 * ORDER,), 2.0 / NS, np.float32)])
    s = np.maximum(y @ B.T, 0.0)
    return (s @ B) * scale[None, None, :]


def _rotate_np(xe, cs):
    """xe: [E,16,5] f32, cs: (c1,s1,c2,s2) each [E]. Returns [E,80] f32."""
    out = np.empty_like(xe)
    out[:, :, 0] = xe[:, :, 0]
    c1, s1, c2, s2 = cs
    out[:, :, 1] = c1[:, None] * xe[:, :, 1] - s1[:, None] * xe[:, :, 2]
    out[:, :, 2] = s1[:, None] * xe[:, :, 1] + c1[:, None] * xe[:, :, 2]
    out[:, :, 3] = c2[:, None] * xe[:, :, 3] - s2[:, None] * xe[:, :, 4]
    out[:, :, 4] = s2[:, None] * xe[:, :, 3] + c2[:, None] * xe[:, :, 4]
    return out.reshape(-1, CQ)


def _prep_topology(edge_index):
    """Sort edges by dst, partition nodes across cores, greedy node-aligned
    128-edge tiles with <= NL nodes each."""
    key = hash(edge_index.tobytes())
    if key in _PREP_CACHE:
        return _PREP_CACHE[key]
    dst = edge_index[:, 1].astype(np.int64)
    src = edge_index[:, 0].astype(np.int64)
    sort_perm = np.argsort(dst, kind="stable")
    dst_s = dst[sort_perm]
    src_s = src[sort_perm]
    deg = np.bincount(dst, minlength=N_NODES)
    estart = np.zeros(N_NODES + 1, np.int64)
    estart[1:] = np.cumsum(deg)

    cores = []
    for ci in range(NCORES):
        n0 = ci * NPC
        tiles = []
        n = n0
        while n < n0 + NPC:
            nn = 0
            ne = 0
            while (n + nn < n0 + NPC and nn < NL
                   and ne + deg[n + nn] <= EC):
                ne += deg[n + nn]
                nn += 1
            assert nn > 0, "node with degree > 128"
            tiles.append((n, nn, estart[n], ne))
            n += nn
        cores.append(tiles)

    T = max(len(t) for t in cores)
    T = ((T + BATCH_T - 1) // BATCH_T) * BATCH_T

    per_core = []
    for ci in range(NCORES):
        tiles = cores[ci]
        slot_edge = np.full(T * EC, -1, np.int64)
        slot_dloc = np.zeros(T * EC, np.int64)
        slot_node = np.full(T * NL, -1, np.int64)
        for t, (nst, nn, est, ne) in enumerate(tiles):
            slot_edge[t * EC:t * EC + ne] = np.arange(est, est + ne)
            slot_dloc[t * EC:t * EC + ne] = dst_s[est:est + ne] - nst
            slot_node[t * NL:t * NL + nn] = np.arange(nst, nst + nn)
        valid = slot_edge >= 0
        per_core.append(dict(slot_edge=slot_edge, slot_dloc=slot_dloc,
                             slot_node=slot_node, valid=valid))
    prep = dict(T=T, sort_perm=sort_perm, src_s=src_s, per_core=per_core)
    _PREP_CACHE[key] = prep
    return prep


def _build_S_inputs(prep, pre_s):
    """Static per-core S-build inputs (shared by both convs):
    - sdm: dense packed S for DENSE_B batches [EC, 4*BATCH_T*SW] bf16
    - preS: pre for SCAT_B batches [EC, 3*BATCH_T*FR] bf16
    - sidx: scatter indices for SCAT_B batches [EC, 3*BATCH_T*FR] i16
    """
    key = id(prep)
    if key in _SD_CACHE:
        return _SD_CACHE[key]
    T = prep["T"]
    out = []
    for pc in prep["per_core"]:
        v = pc["valid"]
        # full dense S, then slice the DENSE_B batches
        S = np.zeros((T * EC, SW), np.float32)
        cols = (pc["slot_dloc"][v] * FR)[:, None] + np.arange(FR)[None, :]
        S[np.nonzero(v)[0][:, None], cols] = pre_s[pc["slot_edge"][v]]
        S = S.reshape(T, EC, SW)
        sdm = np.concatenate(
            [S[b * BATCH_T:(b + 1) * BATCH_T] for b in DENSE_B], axis=0)
        sdm = np.ascontiguousarray(
            sdm.transpose(1, 0, 2).reshape(EC, len(DENSE_B) * BATCH_T * SW)
        ).astype(BF16)
        # pre for scattered batches
        pv = np.zeros((T * EC, FR), np.float32)
        pv[v] = pre_s[pc["slot_edge"][v]]
        pv = pv.reshape(T, EC, FR)
        preS = np.concatenate(
            [pv[b * BATCH_T:(b + 1) * BATCH_T] for b in SCAT_B], axis=0)
        preS = np.ascontiguousarray(
            preS.transpose(1, 0, 2).reshape(EC, len(SCAT_B) * BATCH_T * FR)
        ).astype(BF16)
        # scatter idx for SCAT_B batches
        ix = np.full((T * EC, FR), -1, np.int64)
        slot_t = np.arange(T * EC) // EC
        base = (slot_t % BATCH_T) * SW
        ix[v] = (base[v] + pc["slot_dloc"][v] * FR)[:, None] + np.arange(FR)[None, :]
        ix = ix.reshape(T, EC, FR)
        six = np.concatenate(
            [ix[b * BATCH_T:(b + 1) * BATCH_T] for b in SCAT_B], axis=0)
        six = six.transpose(1, 0, 2).reshape(EC, len(SCAT_B) * BATCH_T * FR)
        out.append((sdm, preS, np.ascontiguousarray(six).astype(np.int16)))
    _SD_CACHE[key] = out
    return out


def _pack_xt(prep, xt_s):
    """xt_s: [E,80] f32 in sorted-edge order -> per-core [128, T*80] bf16."""
    T = prep["T"]
    out = []
    for pc in prep["per_core"]:
        xt = np.zeros((T * EC, CQ), np.float32)
        v = pc["valid"]
        xt[v] = xt_s[pc["slot_edge"][v]]
        xt = xt.reshape(T, EC, CQ).transpose(1, 0, 2).reshape(EC, T * CQ)
        out.append(np.ascontiguousarray(xt).astype(BF16))
    return out


def _pack_params(prep, W, Ws, y_in):
    """w [80,800] | ws [80,80] | xtp [80,T*NL] merged -> [80, 880+T*NL] bf16."""
    T = prep["T"]
    w2 = np.ascontiguousarray(
        W.transpose(1, 3, 4, 5, 0, 2).reshape(CQ, FR * OP)).astype(BF16)
    ws2 = np.ascontiguousarray(
        Ws.transpose(1, 3, 0, 2).reshape(CQ, OP)).astype(BF16)
    out = []
    for pc in prep["per_core"]:
        xtp = np.zeros((T * NL, CQ), np.float32)
        sn = pc["slot_node"]
        v = sn >= 0
        xtp[v] = y_in[sn[v]]
        out.append(np.ascontiguousarray(
            np.concatenate([w2, ws2, xtp.T.astype(BF16)], axis=1)))
    return out


def _unpack_y(prep, youts):
    y = np.empty((N_NODES, CQ), np.float32)
    for ci, pc in enumerate(prep["per_core"]):
        sn = pc["slot_node"]
        v = sn >= 0
        y[sn[v]] = youts[ci].T[v].astype(np.float32)
    return y


# ----------------------------------------------------------------------------
# device program
# ----------------------------------------------------------------------------

def _build_nc(T):
    if T in _NC_CACHE:
        return _NC_CACHE[T]
    nc = bacc.Bacc(None, target_bir_lowering=False)
    NSB = T // BATCH_T
    PW = FR * OP + OP + T * NL
    chunks = []
    t0 = 0
    while t0 < T:
        t1 = min(t0 + CHUNK_T, T)
        chunks.append((t0, t1))
        t0 = t1

    with tile.TileContext(nc) as tc:
        with tc.tile_pool(name="dram", bufs=1, space="DRAM") as dram:
            xt_d = dram.tile([EC, T * CQ], _bf16, kind="ExternalInput", name="xt", uniquify=False)
            sdm_d = dram.tile([EC, len(DENSE_B) * BATCH_T * SW], _bf16, kind="ExternalInput", name="sdm", uniquify=False)
            preS_d = dram.tile([EC, len(SCAT_B) * BATCH_T * FR], _bf16, kind="ExternalInput", name="preS", uniquify=False)
            sidx_d = dram.tile([EC, len(SCAT_B) * BATCH_T * FR], mybir.dt.int16, kind="ExternalInput", name="sidx", uniquify=False)
            pm_d = dram.tile([CQ, PW], _bf16, kind="ExternalInput", name="pm", uniquify=False)
            be_d = dram.tile([OP, 1], _f32, kind="ExternalInput", name="be", uniquify=False)
            y_d = dram.tile([OP, T * NL], _bf16, kind="ExternalOutput", name="y", uniquify=False)

            with tc.tile_pool(name="sb", bufs=1) as sb, \
                 tc.tile_pool(name="ps", bufs=8, space="PSUM") as psA:
                psY = psA
                s_sb = [sb.tile([EC, BATCH_T * SW], _bf16, tag=f"sb{b}",
                                name=f"sbt{b}") for b in range(NSB)]
                xt_sb = sb.tile([EC, T * CQ], _bf16, tag="xts", name="xts")
                preS_sb = sb.tile([EC, len(SCAT_B) * BATCH_T * FR], _bf16)
                sidx_sb = sb.tile([EC, len(SCAT_B) * BATCH_T * FR],
                                  mybir.dt.int16)
                pm_sb = sb.tile([CQ, PW], _bf16)
                be_sb = sb.tile([OP, 1], _f32)
                w_sb = pm_sb[:, 0:FR * OP]
                ws_sb = pm_sb[:, FR * OP:FR * OP + OP]
                xtp_off = FR * OP + OP

                # DMA plan: two serial queues (~1.6us/DMA overhead observed),
                # earliest-needed first; sub-tile deps unblock per range.
                # sync:   sd01 sidx sd3 xt34 pm [y-outs]
                # scalar: xt0 preS xt1 xt2 sd5 xt56 be
                BS = BATCH_T * SW
                BC = BATCH_T * CQ
                def dma_sdm(i0, i1, eng):
                    # dense batches DENSE_B[i0:i1] -> their s_sb tiles; the
                    # sdm tensor packs DENSE_B contiguously
                    for i in range(i0, i1):
                        eng.dma_start(out=s_sb[DENSE_B[i]],
                                      in_=sdm_d[:, i * BS:(i + 1) * BS])
                def dma_xt(c0, c1, eng):
                    eng.dma_start(out=xt_sb[:, c0 * BC:c1 * BC],
                                  in_=xt_d[:, c0 * BC:c1 * BC])
                dma_sdm(0, 2, nc.sync)        # b0+b1 (two issues, back-to-back)
                dma_xt(0, 1, nc.scalar)
                nc.sync.dma_start(out=sidx_sb, in_=sidx_d[:])
                nc.scalar.dma_start(out=preS_sb, in_=preS_d[:])
                dma_xt(1, 2, nc.scalar)
                dma_sdm(2, 3, nc.sync)        # b3
                dma_xt(2, 3, nc.scalar)
                dma_xt(3, 5, nc.sync)
                dma_sdm(3, 4, nc.scalar)      # b5
                nc.sync.dma_start(out=pm_sb, in_=pm_d[:])
                dma_xt(5, 7, nc.scalar)
                nc.scalar.dma_start(out=be_sb, in_=be_d[:])

                # gpsimd scatters for SCAT_B
                for i, b in enumerate(SCAT_B):
                    nc.gpsimd.local_scatter(
                        s_sb[b][:, :],
                        preS_sb[:, i * BATCH_T * FR:(i + 1) * BATCH_T * FR],
                        sidx_sb[:, i * BATCH_T * FR:(i + 1) * BATCH_T * FR],
                        channels=EC, num_elems=BATCH_T * SW,
                        num_idxs=BATCH_T * FR)

                staged = [sb.tile([CQ, (t1 - t0) * SW], _bf16, tag=f"stg{i}",
                                  name=f"stg{i}")
                          for i, (t0, t1) in enumerate(chunks)]
                y_sb = sb.tile([OP, T * NL], _bf16)

                def emit_sgroup(g):
                    ps = psA.tile([CQ, GT * SW], _f32, tag="ps", name=f"psA{g}")
                    for k in range(GT):
                        t = g * GT + k
                        bt, boff = t // BATCH_T, t % BATCH_T
                        lhsT = xt_sb[:, t * CQ:(t + 1) * CQ]
                        rhs = s_sb[bt][:, boff * SW:(boff + 1) * SW]
                        nc.tensor.matmul(ps[:, k * SW:(k + 1) * SW], lhsT, rhs,
                                         start=True, stop=True)
                    ci = (g * GT) // CHUNK_T
                    t0c, t1c = chunks[ci]
                    nt = t1c - t0c
                    gt = g * GT - t0c
                    stg = staged[ci]
                    dst_ap = bass.AP(tensor=stg.tensor,
                                     offset=stg.offset + gt * NL,
                                     ap=[list(stg.ap[0]), [nt * NL, FR],
                                         [NL, GT], [1, NL]])
                    src_ap = bass.AP(tensor=ps.tensor, offset=ps.offset,
                                     ap=[list(ps.ap[0]), [1, FR],
                                         [SW, GT], [FR, NL]])
                    # staging alternates DVE/ACT (gpsimd cannot read PSUM)
                    if g % 2 == 1:
                        nc.scalar.activation(dst_ap, src_ap,
                                             mybir.ActivationFunctionType.Copy)
                    else:
                        nc.vector.tensor_copy(dst_ap, src_ap)

                def emit_node(ci):
                    t0, t1 = chunks[ci]
                    nt = t1 - t0
                    psy = psY.tile([OP, nt * NL], _f32, tag="ps", name=f"psY{ci}")
                    stg = staged[ci]
                    for fr in range(FR):
                        rhs = stg[:, fr * nt * NL:(fr + 1) * nt * NL]
                        nc.tensor.matmul(psy, w_sb[:, fr * OP:(fr + 1) * OP], rhs,
                                         start=(fr == 0), stop=False)
                    nc.tensor.matmul(psy, ws_sb[:, :],
                                     pm_sb[:, xtp_off + t0 * NL:xtp_off + t1 * NL],
                                     start=False, stop=True)
                    nc.scalar.activation(y_sb[:, t0 * NL:t1 * NL], psy,
                                         mybir.ActivationFunctionType.Identity,
                                         bias=be_sb[:, 0:1])
                    nc.sync.dma_start(out=y_d[:, t0 * NL:t1 * NL],
                                      in_=y_sb[:, t0 * NL:t1 * NL])

                NCHK = len(chunks)
                for ci, (t0c, t1c) in enumerate(chunks):
                    for g in range(t0c // GT, t1c // GT):
                        emit_sgroup(g)
                    if ci >= 1:
                        emit_node(ci - 1)
                emit_node(NCHK - 1)
    nc.compile()
    _NC_CACHE[T] = nc
    return nc


# ----------------------------------------------------------------------------
# conv driver
# ----------------------------------------------------------------------------

def _conv_device(prep, S_in, xt_s, y_in, W, Ws, b, trace=False):
    T = prep["T"]
    nc = _build_nc(T)
    be = np.zeros((OP, 1), np.float32)
    be[::Q, 0] = b
    xts = _pack_xt(prep, xt_s)
    pms = _pack_params(prep, W, Ws, y_in)
    in_maps = []
    for ci in range(NCORES):
        in_maps.append({
            "xt": xts[ci], "sdm": S_in[ci][0], "preS": S_in[ci][1],
            "sidx": S_in[ci][2], "pm": pms[ci], "be": be,
        })
    res = run_bass_kernel_spmd(nc, in_maps, core_ids=list(range(NCORES)),
                               trace=trace)
    if res.exec_time_ns is not None:
        LAST_EXEC_NS.append(res.exec_time_ns)
    return _unpack_y(prep, [res.results[ci]["y"] for ci in range(NCORES)])


def kernel(x, edge_index, precomp_neigh_edge, connection, W1, b1, Ws1, W2, b2, Ws2):
    x = np.asarray(x, np.float32)
    ei = np.asarray(edge_index)
    pre = np.asarray(precomp_neigh_edge, np.float32).reshape(N_EDGES, FR)
    phi = np.asarray(connection, np.float32)
    trace = bool(os.environ.get("BASS_TRACE"))
    LAST_EXEC_NS.clear()

    prep = _prep_topology(ei)
    sp = prep["sort_perm"]
    src_s = prep["src_s"]
    pre_s = pre[sp]
    phi_s = phi[sp]
    cs = (np.cos(phi_s), np.sin(phi_s), np.cos(2 * phi_s), np.sin(2 * phi_s))
    cs = tuple(a.astype(np.float32) for a in cs)
    S_in = _build_S_inputs(prep, pre_s)

    # conv1
    xt1 = _rotate_np(x[src_s], cs)
    y1 = _conv_device(prep, S_in, xt1, x.reshape(N_NODES, CQ),
                      np.asarray(W1), np.asarray(Ws1),
                      np.asarray(b1, np.float32), trace)
    y1 = _nonlin_np(y1.reshape(N_NODES, C, Q)).astype(np.float32)

    # conv2
    xt2 = _rotate_np(y1[src_s], cs)
    y2 = _conv_device(prep, S_in, xt2, y1.reshape(N_NODES, CQ),
                      np.asarray(W2), np.asarray(Ws2),
                      np.asarray(b2, np.float32), trace)
    y2 = y2.reshape(N_NODES, C, Q) + x
    return _nonlin_np(y2).astype(np.float32)
